# revision 34
# baseline (speedup 1.0000x reference)
"""Trainium2 Bass kernel for nn_CenterContrastiveLoss.

Problem: loss = label-smoothed CE over [pos, top-50 negs] of f @ centers.T
  f: [2048, 256] f32, centers: [65536, 256] f32, label: [2048] int.

Strategy (8 NeuronCores, tensor-parallel over C=65536):
  - PSUM eviction is the wall (only ScalarE+VectorE can read PSUM, 1 f32/
    cycle each), so the candidate reduction is moved INTO the matmul:
    the host pre-pairs adjacent centers (c_2j + c_2j+1, summed in f32,
    quantized fp8 e4m3), and each core runs DoubleRow matmuls (K=256 in one
    MM, 2x bf16 MAC rate) of fp8 f against the 4096 paired centers of its
    shard. The PE emits pair-SUM screening values directly; eviction is a
    single fp8 copy per [128,1024] PSUM tile, alternating ScalarE/VectorE
    (~37us of copy work per engine, ~28us of matmul).
  - A pair-sum carries the partner column as N(0,16) noise, so the host
    screens deep: top-1536 pair-sums per row, recomputes those 3072 columns
    exactly in f32, excludes the positive, and evaluates
      loss = mean(0.9102*lse([pos, top50]) - 0.9002*pos - 0.0002*sum(top50)).
    Simulated end-to-end rel err: 9.9e-4 (tolerance 2e-2).
"""

import numpy as np
import ml_dtypes

B, C, D = 2048, 65536, 256
NCORES = 8
CSH = C // NCORES          # 8192 original cols per core
RT = B // 128              # 16
NG = 4                     # groups per core
GW = 2048                  # original cols per group
PW = 1024                  # pair-sum cols per group (= stage slot width)
M2 = 1536                  # top pair-sums recomputed exactly per row

_prog = None


def _build_program():
    import concourse.mybir as mybir
    from concourse import bacc
    from concourse.tile import TileContext
    from contextlib import ExitStack

    f32 = mybir.dt.float32
    fp8 = mybir.dt.float8e4
    DR = mybir.MatmulPerfMode.DoubleRow

    nc = bacc.Bacc("TRN2")
    fT_d = nc.declare_dram_parameter("fT", [1, 128, 2, B], fp8, isOutput=False)
    cT_d = nc.declare_dram_parameter("cT", [1, 128, 2, NG * PW], fp8,
                                     isOutput=False)
    out_d = nc.declare_dram_parameter("out", [NG, 128, RT * PW], fp8,
                                      isOutput=True)

    with TileContext(nc) as tc, ExitStack() as ctx:
        const = ctx.enter_context(tc.tile_pool(name="const", bufs=1))
        ctp = ctx.enter_context(tc.tile_pool(name="ctp", bufs=2))
        psum = ctx.enter_context(tc.tile_pool(name="psum", bufs=4,
                                              space="PSUM"))
        stp = ctx.enter_context(tc.tile_pool(name="stp", bufs=2))

        fT_t = const.tile([128, 2, B], fp8, tag="fT", name="fT")

        ct_tiles = []
        for g in range(2):
            ct_tiles.append(ctp.tile([128, 2, PW], fp8, tag="ct",
                                     name=f"ct{g}"))
        # prefetch in need-order across both HWDGE queues
        nc.sync.dma_start(out=ct_tiles[0][:], in_=cT_d[0, :, :, 0:PW])
        nc.scalar.dma_start(out=fT_t[:, :, 0:128], in_=fT_d[0, :, :, 0:128])
        nc.scalar.dma_start(out=fT_t[:, :, 128:B], in_=fT_d[0, :, :, 128:B])
        nc.sync.dma_start(out=ct_tiles[1][:], in_=cT_d[0, :, :, PW:2 * PW])

        cp = mybir.ActivationFunctionType.Copy
        for g in range(NG):
            ct = ct_tiles[g]
            if g + 2 < NG:
                ct_tiles.append(ctp.tile([128, 2, PW], fp8, tag="ct",
                                         name=f"ct{g + 2}"))
            stage = stp.tile([128, RT * PW], fp8, tag="stage", name=f"st{g}")
            for rt in range(RT):
                lhsT = fT_t[:, :, rt * 128:(rt + 1) * 128]
                off = rt * PW
                pt = psum.tile([128, PW], f32, tag="pt", name="pt")
                for j in range(4):
                    nc.tensor.matmul(
                        pt[:, j * 256:(j + 1) * 256], lhsT,
                        ct[:, :, j * 256:(j + 1) * 256],
                        start=True, stop=True, perf_mode=DR)
                if (g * RT + rt) % 2 == 0:
                    nc.scalar.activation(out=stage[:, off:off + PW],
                                         in_=pt[:], func=cp, scale=1.0)
                else:
                    nc.vector.tensor_copy(stage[:, off:off + PW], pt[:])
                # prefetch next-next group once its buffer frees
                if rt == 1 and g + 2 < NG:
                    nc.sync.dma_start(
                        out=ct_tiles[g + 2][:],
                        in_=cT_d[0, :, :, (g + 2) * PW:(g + 3) * PW])
                # batched out-DMA: per-slot DMAs serialize on the Sync
                # sequencer (~0.6us dispatch each) and fall behind eviction.
                # The very last batch is split 2+2 so the final transfer
                # (serial with the kernel tail) is 512KB instead of 1MB.
                last4 = g == NG - 1 and rt >= RT - 4
                if last4 and rt % 2 == 1:
                    nc.sync.dma_start(
                        out=out_d[g, :, (rt - 1) * PW:(rt + 1) * PW],
                        in_=stage[:, (rt - 1) * PW:(rt + 1) * PW])
                elif not last4 and rt % 4 == 3:
                    nc.sync.dma_start(
                        out=out_d[g, :, (rt - 3) * PW:(rt + 1) * PW],
                        in_=stage[:, (rt - 3) * PW:(rt + 1) * PW])

    nc.finalize()
    return nc


def _get_program():
    global _prog
    if _prog is None:
        _prog = _build_program()
    return _prog


def run_device(in_maps, trace=False, **kw):
    from concourse.bass_utils import run_bass_kernel_spmd

    nc = _get_program()
    return run_bass_kernel_spmd(nc, in_maps, core_ids=list(range(NCORES)),
                                trace=trace, **kw)


def make_in_maps(f, centers, label):
    f8 = ml_dtypes.float8_e4m3
    # fT[p, i, b] = f[b, p + 128*i]
    fq = f.astype(f8)                       # [B, 256]
    fT = np.ascontiguousarray(
        fq.T.reshape(2, 128, B).transpose(1, 0, 2)).reshape(1, 128, 2, B)
    in_maps = []
    for core in range(NCORES):
        sh = centers[core * CSH:(core + 1) * CSH]          # [8192, 256] f32
        cq = (sh[0::2] + sh[1::2]).astype(f8)              # [4096, 256]
        cT = np.ascontiguousarray(
            cq.T.reshape(2, 128, NG * PW).transpose(1, 0, 2)).reshape(
                1, 128, 2, NG * PW)
        in_maps.append({"fT": fT, "cT": cT})
    return in_maps


def postprocess(results, f, centers, label):
    f32f = f.astype(np.float32)
    # cand[rt*128+p, (core*NG+g)*PW + j] = out[g, p, rt*PW + j]
    cands = []
    for r in results:
        o = np.asarray(r["out"]).astype(np.float16)  # fp8 -> f16 widen
        o = o.reshape(NG, 128, RT, PW)
        cands.append(o.transpose(2, 1, 0, 3).reshape(B, NG * PW))
    cand = np.concatenate(cands, axis=1).astype(np.float32)  # [B, 32768]
    # candidate j of block (core,g) is the pair (base+2j, base+2j+1)
    blk = np.arange(NCORES * NG)
    pcol = (blk[:, None] // NG * CSH + blk[:, None] % NG * GW
            + 2 * np.arange(PW)).ravel()                     # [32768]

    i2 = np.argpartition(-cand, M2 - 1, axis=1)[:, :M2]      # [B, M2]
    cols = np.concatenate([pcol[i2], pcol[i2] + 1], axis=1)  # [B, 2*M2]
    # exact recompute, chunked over rows to bound the gather working set
    centers_f = centers.astype(np.float32)
    Se = np.empty(cols.shape, np.float64)
    for a in range(0, B, 256):
        b = a + 256
        Se[a:b] = np.einsum('bd,bkd->bk', f32f[a:b], centers_f[cols[a:b]],
                            optimize=True)
    Se[cols == label[:, None]] = -np.inf
    top50 = -np.sort(-Se, axis=1)[:, :50]
    pos = np.einsum('bd,bd->b', f.astype(np.float64),
                    centers[label].astype(np.float64))
    preds = np.concatenate([pos[:, None], top50], axis=1)
    m = preds.max(axis=1, keepdims=True)
    lse = (m + np.log(np.exp(preds - m).sum(axis=1, keepdims=True)))[:, 0]
    S1 = top50.sum(axis=1)
    loss = (0.9102 * lse - 0.9002 * pos - 0.0002 * S1).mean()
    return np.array(loss, dtype=np.float32)


def kernel(f, centers, label):
    f = np.asarray(f, dtype=np.float32)
    centers = np.asarray(centers, dtype=np.float32)
    label = np.asarray(label).astype(np.int64)
    in_maps = make_in_maps(f, centers, label)
    try:
        res = run_device(in_maps)
    except Exception:
        # transient runtime flakes (e.g. NRT_EXEC_UNIT_UNRECOVERABLE) have
        # been observed to succeed on immediate retry
        res = run_device(in_maps)
    return postprocess(res.results, f, centers, label)
